# revision 79
# baseline (speedup 1.0000x reference)
"""AttentionMixer kernel for 8 Trainium2 NeuronCores.

Sharding: data-parallel over (batch B=4) x (query-half NQ/2) -> 8 cores.
Each core computes, for its (b, half):
    q = meshT slice proj, k/v = pc proj (k/v work duplicated across the
    2 cores of a batch), masked softmax attention, Wo projection.
Layout is "transposed" throughout (features on partitions, tokens on the
free dim) so every matmul contracts over the partition dim natively:
    qT/kT: [e, n] via W.T as lhsT, xT as rhs (bf16)
    scoresT: [nk, nq] = kT_h.T-contract-d qT_h  (2 heads row-packed)
    attnT = exp(scoresT/8 + mask) in fp8e4m3, produced ALTERNATELY by
      - ScalarE: native Exp ACTIVATE (handles masked/partial blocks)
      - VectorE: Schraudolph bit-trick -- one tensor_scalar op computes
        int8(round(1.4427*s + 55.63)) whose int8 bits ARE the e4m3
        encoding of exp(s/8) to ~7% per-element (averages out over the
        ~2500-key softmax; tolerance is 2e-2).  Only fully-valid blocks
        (j < jfull on every core) go to VectorE so no mask is needed.
    ctxT_h: [65, nq] via fp8 DoubleRow matmuls -- one matmul contracts
      TWO 128-key blocks (virtual K=256), with a ones column appended to
      v giving the softmax denominator Z for free.  mix = ctx/Z @ Wo.T.
Projections/scores bf16 with fp32 PSUM accumulation; inputs pcT/meshT
are fp8 (error averages out through the 128-dim projections).

The j-loop is software-pipelined at block-pair granularity: scores+exp
of pair p are emitted before ctx of pair p-1 so PE never stalls behind
the current exp, and the two exp engines run concurrently.

Scheduling notes (trace-driven):
  - Startup: input DMAs are split over the sync-HWDGE and gpsimd-SWDGE
    queues (one queue serializes issues at ~0.65us each and completion
    sems fire ~2.5us after issue); NEVER the scalar queue -- its DMAs
    raced the ACT_TABLE_LOAD and intermittently corrupted loads.  The
    HAM warm-up matmul count is tuned to the first DMA-sem arrival.
  - Pass-end normalization: one merged [1,1024] reciprocal for both
    heads; broadcast AND the ctx multiply run on the otherwise-idle
    GpSimd so ACT/DVE stay on exp.  ScalarE's exp share starts at j=3
    because it does the staging copies at each pass start.
  - Final pass: pairs holding masked (ACT-forced) blocks run mid-pass
    so the last exps alternate engines; the tail norm chain skips
    staging (multiplies read the accumulators in PSUM directly) and
    filler matmuls keep the PE clock un-throttled through it.  mixT
    exits via DVE+sync-DMA (eb0) and ScalarE+scalar-DMA (eb1) in
    parallel, with eb1's bias folded into a rank-1 bf16 matmul.
"""

import math

import numpy as np
import ml_dtypes

import concourse.bass as bass
import concourse.bacc as bacc
import concourse.mybir as mybir
import concourse.tile as tile
from concourse.bass_utils import run_bass_kernel_spmd

B, NQ, NK, E, DPC, H = 4, 2048, 4096, 256, 128, 4
HD = E // H  # 64
NQH = NQ // 2  # per-core queries: 1024
NKB = NK // 128  # 32 nk blocks
P = 128
BF16 = mybir.dt.bfloat16
F32 = mybir.dt.float32
F8 = mybir.dt.float8e4
I8 = mybir.dt.int8
MASK_NEG = -80.0
# Schraudolph exp in e4m3-bit space: bits = round(SLOPE*s + CCONST)
# (DVE fp32->int8 convert is round-half-even, probed on HW).
SLOPE = 8 * math.log2(math.e) * 0.125   # 1.4427
CCONST = 55.63
VPAD = 80  # per-head stride in v_sb (65 used; *H must be 16B aligned)

_CACHE = {}


def build_nc(jmax=NKB, jfull=NKB):
    nc = bacc.Bacc(None)
    knt = (jmax + 3) // 4         # 512-wide kT tiles needed
    nch = (jmax + 7) // 8         # 1024-wide pcT DMA chunks needed
    npair = (jmax + 1) // 2       # ctx block-pairs (last may be single)
    odd = jmax % 2 == 1

    # blocks that must use ScalarE (need the mask bias); plus alternating
    # leading blocks so the two exp engines get ~equal work
    tail = list(range(jfull, jmax))
    n_act_extra = max(0, (jmax + 1) // 2 - len(tail))
    # ScalarE's share starts at j=3: at each pass start ScalarE is busy
    # with the previous pass's staging copies, so the first blocks go to
    # the (then-idle) VectorE
    act_blocks = set(tail) | set(range(jfull)[3::2][:n_act_extra])

    # ---- DRAM params (per-core shapes; host stages exact SBUF layouts) ----
    meshT_d = nc.declare_dram_parameter("meshT", [P, 2, NQH], F8, False)
    pcT_d = nc.declare_dram_parameter("pcT", [P, NK], F8, False)
    wqT_d = nc.declare_dram_parameter("wqT", [P, 2, E], BF16, False)
    # aux row on partition 0: [bop(eb1) 128 | ones 512] for the bias
    # outer-product matmul (ACT's Copy can't take an AP bias); bf16 so
    # the matmul doesn't pay the fp32 LOW/HIGH two-pass cost
    aux_d = nc.declare_dram_parameter("aux", [1, 640], BF16, False)
    wkT_d = nc.declare_dram_parameter("wkT", [P, E], BF16, False)
    wvT_d = nc.declare_dram_parameter("wvT", [P, E], BF16, False)
    woT_d = nc.declare_dram_parameter("woT", [HD, H, E], BF16, False)
    # consts: [bk | bq | bop | maskb] along the free dim
    consts_d = nc.declare_dram_parameter("consts", [P, 6 + NKB], F32, False)
    mixT_d = nc.declare_dram_parameter("mixT", [2, P, NQH], BF16, isOutput=True)

    with tile.TileContext(nc) as tc:
        with (
            tc.tile_pool(name="const", bufs=1) as cpool,
            tc.tile_pool(name="acts", bufs=1) as apool,
            tc.tile_pool(name="attn", bufs=4) as attn_pool,
            tc.tile_pool(name="small", bufs=2) as spool,
            tc.tile_pool(name="ps_big", bufs=3, space="PSUM") as ps_big,
            tc.tile_pool(name="ps_ctx", bufs=2, space="PSUM") as ps_ctx,
        ):
            # ---- load constants / inputs into SBUF ----
            meshT = cpool.tile([P, 2, NQH], F8)
            pcT = cpool.tile([P, NK], F8)
            wqT = cpool.tile([P, 2, E], BF16)
            aux = cpool.tile([1, 640], BF16)
            wkT = cpool.tile([P, E], BF16)
            wvT = cpool.tile([P, E], BF16)
            woT = cpool.tile([HD, H, E], BF16)
            consts = cpool.tile([P, 6 + NKB], F32)
            # consts[:, 0:2] holds bk, unused: the k-bias adds a per-query
            # constant to every key's score, which softmax cancels.
            bq = consts[:, 2:4]
            bop = consts[:, 4:6]
            maskb = consts[:, 6:6 + NKB]

            # warm tile memset as GpSimd's first op (its queue drains its
            # preamble ~1us before DVE's) so warm-up matmuls start ASAP
            warm = cpool.tile([P, 512], BF16)
            nc.gpsimd.memset(warm[:], 0.0)

            # critical-path inputs split over the sync (HWDGE) and gpsimd
            # (SWDGE) DMA queues: each dma_start occupies its issuing
            # queue ~0.65us and its completion semaphore fires ~2.5us
            # after issue, so one queue would serialize the startup.
            # (NOT the scalar queue: its first slot belongs to the
            # ACT_TABLE_LOAD and input DMAs there raced it.)
            nc.sync.dma_start(wkT[:], wkT_d[:, :])
            nc.sync.dma_start(pcT[:, 0:512], pcT_d[:, 0:512])
            nc.sync.dma_start(wqT[:], wqT_d[:, :, :])
            nc.sync.dma_start(pcT[:, 512:1024], pcT_d[:, 512:1024])
            nc.gpsimd.dma_start(consts[:], consts_d[:, :])
            nc.gpsimd.dma_start(meshT[:, 0, :], meshT_d[:, 0, :])
            nc.gpsimd.dma_start(meshT[:, 1, :], meshT_d[:, 1, :])
            nc.gpsimd.dma_start(wvT[:], wvT_d[:, :])
            nc.gpsimd.dma_start(aux[:], aux_d[:, :])
            for ch in range(1, nch):
                nc.gpsimd.dma_start(pcT[:, ch * 1024:(ch + 1) * 1024],
                                    pcT_d[:, ch * 1024:(ch + 1) * 1024])
            nc.gpsimd.dma_start(woT[:], woT_d[:, :, :])

            # HAM warm-up: dependency-free matmuls during the input-DMA
            # window so the PE clock gate is at 2.4 GHz for the real work.
            # Sized to roughly the input-DMA latency: a longer warm-up
            # would DELAY the first real matmul (PE FIFO is in-order).
            wps = ps_big.tile([P, 1024], F32, tag="big")
            for _ in range(18):
                nc.tensor.matmul(wps[:, 0:256], warm[:, 0:128],
                                 warm[:, 0:256], start=True, stop=True)

            kT = apool.tile([P, 2, knt * 512], BF16)
            qT = apool.tile([P, 2, NQH], BF16)
            # v (fp8) in DoubleRow pair layout + ones column for Z
            v_sb = apool.tile([P, npair, 2, H * VPAD], F8)
            for h in range(H):
                nc.vector.memset(
                    v_sb[:, :, :, h * VPAD + HD:h * VPAD + HD + 1], 1.0)
            mixT = apool.tile([P, 2, NQH], BF16)
            ctxn = apool.tile([HD, H, NQH], BF16)  # normalized ctxT per head

            def k_proj_eb(eb, nt0, n_nt, first_dve=False):
                # n_nt (1 or 2) 512-wide kT tiles for one e-block; the
                # PSUM->SBUF casts are split across ScalarE/VectorE so
                # neither exp engine eats the whole cost
                ps = ps_big.tile([P, 1024], F32, tag="big")
                for i in range(n_nt):
                    nc.tensor.matmul(
                        ps[:, i * 512:(i + 1) * 512],
                        wkT[:, eb * P:(eb + 1) * P],
                        pcT[:, (nt0 + i) * 512:(nt0 + i + 1) * 512],
                        start=True, stop=True,
                    )
                    # kT is pre-scaled by SLOPE so the DVE exp needs only a
                    # single ADD op and ACT exp just rescales
                    dst = kT[:, eb, (nt0 + i) * 512:(nt0 + i + 1) * 512]
                    if i == 0 and not first_dve:
                        nc.scalar.activation(
                            dst, ps[:, 0:512],
                            mybir.ActivationFunctionType.Copy, scale=SLOPE)
                    else:
                        nc.vector.tensor_scalar_mul(
                            dst, ps[:, i * 512:(i + 1) * 512], SLOPE)

            def q_proj_ebnt(eb, nt):
                ps = ps_big.tile([P, 1024], F32, tag="big")
                for cb in range(2):
                    nc.tensor.matmul(
                        ps[:, 0:512],
                        wqT[:, cb, eb * P:(eb + 1) * P],
                        meshT[:, cb, nt * 512:(nt + 1) * 512],
                        start=(cb == 0), stop=(cb == 1),
                    )
                nc.vector.tensor_scalar_add(
                    qT[:, eb, nt * 512:(nt + 1) * 512], ps[:, 0:512],
                    bq[:, eb:eb + 1])

            def v_pair(p, scalar_eng=False):
                # both blocks of a pair -> one ps tile, one engine copy
                # (gpsimd can't help: it has no PSUM access)
                blocks = [2 * p] if odd and p == npair - 1 else [2 * p, 2 * p + 1]
                ps = ps_big.tile([P, 1024], F32, tag="big")
                for ci, j in enumerate(blocks):
                    nc.tensor.matmul(
                        ps[:, ci * E:(ci + 1) * E],
                        pcT[:, j * P:(j + 1) * P],
                        wvT[:],
                        start=True, stop=True,
                    )
                nb = len(blocks)
                vdst = v_sb[:, p, :, :].rearrange(
                    "p c (h x) -> p c h x", x=VPAD)[:, 0:nb, :, 0:HD]
                src = ps[:, 0:nb * E].rearrange(
                    "p (c h x) -> p c h x", c=nb, x=HD)
                if scalar_eng:
                    nc.scalar.activation(
                        vdst, src, mybir.ActivationFunctionType.Copy)
                else:
                    nc.vector.tensor_copy(vdst, src)

            def wo_proj(nt, ebs=(0, 1)):
                # mixT[e'] = sum_h WoT_h.T @ ctxn_h (+ bop).  eb0 exits via
                # DVE + sync-queue DMA, eb1 via ScalarE (bias add in the
                # Copy) + scalar-queue DMA so the two chains overlap.
                for eb in ebs:
                    ps = ps_big.tile([P, 1024], F32, tag="big")
                    for h in range(H):
                        nc.tensor.matmul(
                            ps[:, 0:512],
                            woT[:, h, eb * P:(eb + 1) * P],
                            ctxn[:, h, nt * 512:(nt + 1) * 512],
                            start=(h == 0), stop=(h == H - 1) and eb == 0,
                        )
                    dst = mixT[:, eb, nt * 512:(nt + 1) * 512]
                    if eb == 0:
                        nc.vector.tensor_scalar_add(
                            dst, ps[:, 0:512], bop[:, 0:1])
                        nc.sync.dma_start(
                            mixT_d[0][:, nt * 512:(nt + 1) * 512], dst)
                    else:
                        # bias via rank-1 outer product bop(eb1) x ones
                        nc.tensor.matmul(
                            ps[:, 0:512], aux[:, 0:128], aux[:, 128:640],
                            start=False, stop=True)
                        nc.scalar.activation(
                            dst, ps[:, 0:512],
                            mybir.ActivationFunctionType.Copy)
                        nc.scalar.dma_start(
                            mixT_d[1][:, nt * 512:(nt + 1) * 512], dst)

            def exp_block(j, s, dst):
                # dst: fp8 [P, 1024] plane of the pair tile; scores arrive
                # pre-scaled by SLOPE (folded into kT)
                if j in act_blocks:
                    nc.scalar.activation(
                        dst, s[:],
                        mybir.ActivationFunctionType.Exp,
                        bias=maskb[:, j:j + 1], scale=0.125 / SLOPE)
                else:
                    nc.vector.tensor_scalar_add(
                        dst.bitcast(I8), s[:], CCONST)

            # k tiles 0-1 (pcT chunk 0), then q, then the remaining k
            # tiles; v is interleaved into the first attention pass
            # first k tile's cast on DVE: at startup ScalarE is still busy
            # with the ACT table load while DVE is free
            k_proj_eb(0, 0, 1, first_dve=True)
            q_proj_ebnt(0, 0)
            # prologue v (pairs 0-3) on ScalarE: their copies land in the
            # startup dead zone (ScalarE is idle between the ACT table
            # load and the first exps) instead of crowding pass 0, and
            # the DVE's first exp isn't queued behind them
            v_pair(0, scalar_eng=True)
            if npair > 1:
                v_pair(1, scalar_eng=True)
            if npair > 2:
                v_pair(2, scalar_eng=True)
            if npair > 3:
                v_pair(3, scalar_eng=True)
            ke0 = [("k", 0, nt0, min(2, knt - nt0))
                   for nt0 in range(2, knt, 2)]
            ke1 = [("k", 1, nt0, min(2, knt - nt0))
                   for nt0 in range(0, knt, 2)]
            extras_p0 = [("k", 0, 1, 1)] + ke0 + [ke1[0], ("q", 1, 0, 0)]
            extras_p1 = ke1[1:] + [("q", 0, 1, 0), ("q", 1, 1, 0)]

            # ---- attention main loop (software-pipelined over pairs) ----
            # At pass end the raw ctx+Z rows are staged PSUM->SBUF with ONE
            # ScalarE copy per head ([65,512]: ctx rows + Z row together,
            # instantly freeing the accumulators); the
            # reciprocal/broadcast/multiply tail is DEFERRED into the next
            # pass so pass transitions never serialize the exp engines /
            # idle the PE (which would re-throttle HAM).  Both heads' Z
            # rows share one DVE reciprocal; the broadcast and the
            # normalize multiply run on the otherwise-idle GpSimd engine
            # (all-SBUF operands), keeping ACT/DVE free for exp.
            pending_norm = []
            carry = None
            passes = [(0, 0), (1, 0), (0, 1), (1, 1)]
            for pi, (hp, nt) in enumerate(passes):
                h0, h1 = 2 * hp, 2 * hp + 1
                acc0 = ps_ctx.tile([HD + 1, 512], F32, tag="ctx")
                acc1 = ps_ctx.tile([HD + 1, 512], F32, tag="ctx")
                pend = None

                def ctx_pair(a, p, single, first, last):
                    for acc, h in ((acc0, h0), (acc1, h1)):
                        vsl = v_sb[:, p, :, h * VPAD:h * VPAD + HD + 1]
                        asl = a[:, :, (h % 2) * 512:(h % 2 + 1) * 512]
                        if single:
                            nc.tensor.matmul(
                                acc[:], vsl[:, 0, :], asl[:, 0, :],
                                start=first, stop=last)
                        else:
                            nc.tensor.matmul(
                                acc[:], vsl, asl,
                                start=first, stop=last,
                                perf_mode=mybir.MatmulPerfMode.DoubleRow)

                # final pass: move the pairs holding ScalarE-forced masked
                # tail blocks to mid-pass so the LAST exps alternate
                # engines and the pass doesn't end on a serial ACT run
                pair_order = list(range(npair))
                if pi == len(passes) - 1 and jfull < jmax:
                    masked = [p for p in range(npair) if 2 * p + 1 >= jfull]
                    valid = [p for p in range(npair) if p not in masked]
                    pair_order = valid[:3] + masked + valid[3:]
                for pos, p in enumerate(pair_order):
                    single = odd and p == npair - 1
                    js = [2 * p] if single else [2 * p, 2 * p + 1]
                    a = attn_pool.tile([P, 2, 1024], F8, tag="attn")
                    for idx, j in enumerate(js):
                        s = ps_big.tile([P, 1024], F32, tag="big")
                        # scores for the two heads -> adjacent psum banks;
                        # 64-row lhsT slices on disjoint row groups run
                        # concurrently on the PE
                        nc.tensor.matmul(
                            s[:, 0:512],
                            kT[0:HD, hp, j * P:(j + 1) * P],
                            qT[0:HD, hp, nt * 512:(nt + 1) * 512],
                            start=True, stop=True,
                        )
                        nc.tensor.matmul(
                            s[:, 512:1024],
                            kT[HD:P, hp, j * P:(j + 1) * P],
                            qT[HD:P, hp, nt * 512:(nt + 1) * 512],
                            start=True, stop=True,
                        )
                        exp_block(j, s, a[:, idx, :])
                        if pi == 0 and idx == 0 and 2 <= p < npair - 2:
                            # v pair 2 ahead of ctx; alternate engines
                            v_pair(p + 2, scalar_eng=p % 2 == 0)
                        extras = (extras_p0 if pi == 0 else
                                  extras_p1 if pi == 1 else [])
                        if j % 2 == 1 and (j - 1) // 2 < len(extras):
                            kind, eb, nt0, n_nt = extras[(j - 1) // 2]
                            if kind == "k":
                                k_proj_eb(eb, nt0, n_nt)
                            else:
                                q_proj_ebnt(eb, nt0)
                        if pi == 2 and pos == 7 and idx == 0:
                            wo_proj(0, ebs=(0,))
                        if pi == 3 and pos == 7 and idx == 0:
                            wo_proj(0, ebs=(1,))
                    # zero-contribution matmul (adds 0*0 into the live
                    # accumulator) keeps the PE activity monitor from
                    # re-throttling the clock during exp-wait gaps; in
                    # pass 0's first pairs the exp queues are still
                    # filling (PE ~40% busy) so several are needed or HAM
                    # re-throttles the clock ~3us in
                    if pos % 2 == 1 or pos in (2, 3):
                        nc.tensor.matmul(acc0[:, 0:256], warm[0:P, 0:HD + 1],
                                         warm[:, 0:256],
                                         start=False, stop=False,
                                         skip_group_check=True)
                    if pos == 0 and carry is not None:
                        # previous pass's final ctx + staging, emitted
                        # AFTER this pass's first scores/exps so neither
                        # the PE FIFO nor the ACT queue stalls the new
                        # pass at the transition
                        carry()
                        carry = None
                    elif pos >= 2 and pending_norm:
                        # delayed one extra pair so a popped op's upstream
                        # (e.g. recip's ACT zrow copy) has surely landed
                        # and it never blocks its strict-FIFO engine queue
                        pending_norm.pop(0)()
                    if pend is not None:
                        ctx_pair(*pend, last=False)
                    pend = (a, p, single, pos == 0)

                def make_carry(ctx_pair, pend, acc0, acc1, h0, h1, nt,
                               final=False):
                    def run():
                        nonlocal pending_norm
                        ctx_pair(*pend, last=True)
                        # stage ctx rows + Z rows to SBUF (Z copy is the
                        # one legal cross-partition hop 64 -> 0, ScalarE
                        # only); h0 staged here, h1 deferred one pair and
                        # put on DVE so no single engine eats a 2.7us
                        # burst at the pass transition.  recip/broadcast/
                        # multiply are single merged ops; the multiply
                        # runs on the otherwise-idle GpSimd.
                        stage = spool.tile([HD, 1024], F32, tag="stage")
                        zrow = spool.tile([1, 1024], F32, tag="zrow")
                        zr = spool.tile([1, 1024], F32, tag="zr")
                        zbs = spool.tile([HD, 1024], F32, tag="zbs")
                        for i, acc in ((0, acc0), (1, acc1)):
                            nc.scalar.activation(
                                stage[:, i * 512:(i + 1) * 512],
                                acc[0:HD, :],
                                mybir.ActivationFunctionType.Copy)
                            nc.scalar.activation(
                                zrow[:, i * 512:(i + 1) * 512],
                                acc[HD:HD + 1, :],
                                mybir.ActivationFunctionType.Copy)

                        def phase1():
                            # both heads' 1/Z in one DVE op
                            nc.vector.reciprocal_approx_fast(
                                zr[:], zrow[:])

                        def phase2():
                            nc.gpsimd.partition_broadcast(zbs[:], zr[:])

                        def phase3():
                            sview = stage[:].rearrange(
                                "p (c q) -> p c q", c=2)
                            zview = zbs[:].rearrange("p (c q) -> p c q", c=2)
                            nc.gpsimd.tensor_mul(
                                ctxn[:, h0:h1 + 1, nt * 512:(nt + 1) * 512],
                                sview, zview)

                        pending_norm = [phase1, phase2, phase3]

                    def run_final():
                        # tail variant: no staging (no next pass needs the
                        # accumulators) -- single merged recip/broadcast,
                        # multiplies on DVE straight from PSUM (the gp
                        # tensor_mul is ~3x slower and pays a long drain)
                        nonlocal pending_norm
                        ctx_pair(*pend, last=True)
                        zrow = spool.tile([1, 1024], F32, tag="zrow")
                        zr = spool.tile([1, 1024], F32, tag="zr")
                        zbs = spool.tile([HD, 1024], F32, tag="zbs")
                        for i, acc in ((0, acc0), (1, acc1)):
                            nc.scalar.activation(
                                zrow[:, i * 512:(i + 1) * 512],
                                acc[HD:HD + 1, :],
                                mybir.ActivationFunctionType.Copy)

                        def s_recip():
                            nc.vector.reciprocal_approx_fast(zr[:], zrow[:])

                        def s_bcast():
                            nc.gpsimd.partition_broadcast(zbs[:], zr[:])

                        def s_mul0():
                            nc.vector.tensor_mul(
                                ctxn[:, h0, nt * 512:(nt + 1) * 512],
                                acc0[0:HD, :], zbs[:, 0:512])

                        def s_mul1():
                            nc.vector.tensor_mul(
                                ctxn[:, h1, nt * 512:(nt + 1) * 512],
                                acc1[0:HD, :], zbs[:, 512:1024])

                        pending_norm = [s_recip, s_bcast, s_mul0, s_mul1]
                    return run_final if final else run

                final = pi == len(passes) - 1
                carry = make_carry(ctx_pair, pend, acc0, acc1, h0, h1, nt,
                                   final=final)
                if final:
                    carry()
                    # keep the PE busy through the serial norm tail so HAM
                    # doesn't re-throttle the clock before the wo matmuls
                    fill = ps_big.tile([P, 1024], F32, tag="big")
                    for f in pending_norm:
                        f()
                        for _ in range(7):
                            nc.tensor.matmul(
                                fill[:, 0:512], warm[:, 0:128],
                                warm[:, 0:512], start=True, stop=True)
                    pending_norm = []
            if jmax <= 10:
                wo_proj(0)
            wo_proj(1)

    nc.finalize()
    return nc


def _get_nc(jmax, jfull):
    key = (jmax, jfull)
    if key not in _CACHE:
        _CACHE[key] = build_nc(jmax, jfull)
    return _CACHE[key]


def kernel(mesh_feats, pc_feats, Wq, Wk, Wv, bq, bk, bv, Wo, bo, lengths,
           _trace=False, _trace_kwargs=None):
    mesh_feats = np.asarray(mesh_feats, np.float32)
    pc_feats = np.asarray(pc_feats, np.float32)
    Wq, Wk, Wv = (np.asarray(x, np.float32) for x in (Wq, Wk, Wv))
    bqv, bkv, bvv = (np.asarray(x, np.float32) for x in (bq, bk, bv))
    Wo, bo = np.asarray(Wo, np.float32), np.asarray(bo, np.float32)
    lengths = np.asarray(lengths, np.int32)

    bf = ml_dtypes.bfloat16
    f8 = ml_dtypes.float8_e4m3
    wqT = np.ascontiguousarray(
        Wq.T.reshape(2, P, E).transpose(1, 0, 2)).astype(bf)   # [128, 2, 256]
    wkT = np.ascontiguousarray(Wk.T).astype(bf)          # [128, 256]
    wvT = np.ascontiguousarray(Wv.T).astype(bf)          # [128, 256]
    woT = np.ascontiguousarray(
        Wo.T.reshape(H, HD, E).transpose(1, 0, 2)).astype(bf)  # [64, 4, 256]
    bq2 = np.ascontiguousarray(bqv.reshape(2, P).T)      # [128, 2]
    bk2 = np.ascontiguousarray(bkv.reshape(2, P).T)
    bop = Wo @ bvv + bo
    bop2 = np.ascontiguousarray(bop.reshape(2, P).T)
    aux = np.concatenate(
        [bop[P:2 * P], np.ones(512, np.float32)]).reshape(1, 640)
    aux = np.ascontiguousarray(aux.astype(bf))

    jmax = int(min(NKB, max(1, math.ceil(int(lengths.max()) / 128))))
    jfull = int(min(NKB, int(lengths.min()) // 128))

    idx = np.arange(NK).reshape(NKB, P).T                # [128, 32]
    in_maps = []
    for c in range(8):
        b, half = c // 2, c % 2
        meshT = np.ascontiguousarray(
            mesh_feats[b, half * NQH:(half + 1) * NQH, :].T
            .reshape(2, P, NQH).transpose(1, 0, 2)).astype(f8)  # [128,2,1024]
        pcT = np.ascontiguousarray(pc_feats[b].T).astype(f8)
        maskb = np.where(idx < int(lengths[b]), 0.0, MASK_NEG).astype(np.float32)
        consts = np.ascontiguousarray(
            np.concatenate([bk2, bq2, bop2, maskb], axis=1).astype(np.float32))
        in_maps.append({
            "meshT": meshT, "pcT": pcT, "wqT": wqT, "wkT": wkT,
            "wvT": wvT, "woT": woT, "consts": consts, "aux": aux,
        })

    nc = _get_nc(jmax, jfull)
    res = run_bass_kernel_spmd(
        nc, in_maps, list(range(8)),
        trace=_trace, **(_trace_kwargs or {}))
    out = np.empty((B, NQ, 2 * E), np.float32)
    out[:, :, :E] = mesh_feats
    for c in range(8):
        b, half = c // 2, c % 2
        mixT = res.results[c]["mixT"]            # [2, 128, NQH] bf16
        out[b, half * NQH:(half + 1) * NQH, E:] = \
            mixT.astype(np.float32).reshape(E, NQH).T
    if _trace:
        return out, res
    return out

